# revision 23
# baseline (speedup 1.0000x reference)
"""Detail-loss kernel for TRN2 (8 NeuronCores), v3.

Reference computation (algebraically reduced):
  views = reshape(inputs, (98, 3, 256, 256)); d = infer - ref
  S[n] = sum_c d[n, c]                       (per-view 256x256 plane)
  loss = ( sum |S[n,h,w+1] - S[n,h,w-1]|     (zero-padded outside)
         + sum |S[n,h+1,w] - S[n,h-1,w]| ) / (4 * 98 * 258 * 256)

Sharding: 98 views padded to 104, 13 views per core (zero views add 0).

v3 changes vs v2 (41.8us):
  * DMA rebalanced: views 0-4 on gpsimd(SWDGE), 5-8 on sync, 9-12 on
    scalar queues as 2-view-group transfers (128 x 6KB descriptors).
    v2 put 9/13 views on gpsimd -> 7us single-queue DMA tail.
  * PE warmup: dummy matmuls on junk data at kernel start lift the
    PE_HAM clock gate (1.2 -> 2.4 GHz) before real data arrives, so
    real matmuls run at the 216ns warm cadence instead of ~430ns.
  * gw computed directly from PSUM S on DVE (interior diff + edge-col
    copy), removing the S-copy -> gw chain; S-copy (needed only as the
    gh matmul moving operand) runs in parallel on ACT.
  * per-pair abs-accumulate ops alternate DVE/ACT; some mid-kernel gw
    subtracts route via SBUF on gpsimd to relieve DVE.
Host: sum partials in float64, scale.
"""
import numpy as np
import ml_dtypes
import concourse.bass as bass
import concourse.mybir as mybir
from concourse import bacc
from concourse.tile import TileContext
from concourse.bass_utils import run_bass_kernel_spmd

N_CORES = 8
V = 13                       # views per core (98 -> 104 padded)
C, H, W = 3, 256, 256
SCALE = 1.0 / (4.0 * 98.0 * 258.0 * 256.0)

# x traffic rides ONE DMA ring (gpsimd/SWDGE) in FIFO order: a single
# ring running alone sustains ~358GB/s (HBM cap), while concurrent rings
# round-robin at packet granularity and all complete late (~260GB/s
# aggregate, no useful ordering). Exception: weights + view 0 go on the
# sync ring (HWDGE, faster first-byte; drains before the gpsimd stream
# ramps) so compute starts ~2us earlier. First views go singly for early
# pipeline start; later ones per-pair (fewer Q7 descriptor-gen ops).
# Units are (view_lo, n_views, queue).
DMA_UNITS = [
    (0, 1, "s"), (1, 1, "g"), (2, 1, "g"), (3, 1, "g"),
    (4, 2, "g"), (6, 2, "g"), (8, 2, "g"), (10, 2, "g"), (12, 1, "g"),
]
# processing units: (view_lo, n)
UNITS = [(0, 1), (1, 1), (2, 1), (3, 1), (4, 2), (6, 2), (8, 2), (10, 2), (12, 1)]
NUNIT = len(UNITS)

N_WARM = 7                   # PE warmup matmuls (512 cols, plain mode)

# engine assignment per unit, from measured per-op costs (gpsimd can run
# only plain TENSOR_TENSOR/TENSOR_SCALAR — no PSUM access, no reduce):
#   S-copy: ACT (~1.1ns/elem); gw-sub: gpsimd once its DMA descriptor
#   generation ends (~13us), DVE for the first and last units; gw-abs:
#   DVE reduce; gh-abs: alternate ACT/DVE (DVE at the tail: no
#   READ_ACCUMULATOR step).
COPY_ENG = "aaaaaaaaa"
GWSUB_ENG = "vvggggggv"
GWABS_ENG = "vvvvvvvvv"
GHABS_ENG = "avavavavv"

_cache = {}


def _weights():
    I = np.eye(128, dtype=np.float32)
    E = (np.eye(128) - np.eye(128, k=1)).astype(np.float32)   # out[o]=in[o]-in[o-1]
    O = (np.eye(128, k=-1) - np.eye(128)).astype(np.float32)  # out[o]=in[o+1]-in[o]
    wpair = np.stack([I, -I], axis=1)  # [128, 2, 128] DoubleRow stationary
    weo = np.stack([E, O], axis=1)     # [128, 2, 128]
    return wpair, weo


def _build():
    if "nc" in _cache:
        return _cache["nc"]
    f32 = mybir.dt.float32
    bf16 = mybir.dt.bfloat16
    f8 = mybir.dt.float8e4
    AluOp = mybir.AluOpType
    Act = mybir.ActivationFunctionType
    DR = mybir.MatmulPerfMode.DoubleRow

    nc = bacc.Bacc(None, target_bir_lowering=False)
    x = nc.declare_dram_parameter("x", [128, V, C, 2, 512], f8, isOutput=False)
    wp = nc.declare_dram_parameter("wp", [128, 2, 128], f8, isOutput=False)
    we = nc.declare_dram_parameter("we", [128, 2, 128], bf16, isOutput=False)
    y = nc.declare_dram_parameter("y", [128, 2 * NUNIT], f32, isOutput=True)

    with TileContext(nc) as tc:
        with (
            tc.tile_pool(name="wpool", bufs=1) as wpool,
            tc.tile_pool(name="jp", bufs=1) as jpool,
            tc.tile_pool(name="xp", bufs=1) as xpool,
            tc.tile_pool(name="sp", bufs=3) as spool,
            tc.tile_pool(name="gp", bufs=2) as gpool,
            tc.tile_pool(name="zp", bufs=2) as zpool,
            tc.tile_pool(name="cp", bufs=2) as cpool,
            tc.tile_pool(name="ap", bufs=1) as apool,
            tc.tile_pool(name="psS", bufs=2, space="PSUM") as psSp,
            tc.tile_pool(name="psG", bufs=2, space="PSUM") as psGp,
        ):
            # ---- junk memset first so PE warmup can start immediately
            junk = jpool.tile([128, 512], f8)
            nc.vector.memset(junk[:], 0.0)

            # ---- weight DMAs on sync (own HWDGE ring, completes before x
            # traffic ramps); all x views on the gpsimd ring in FIFO order
            wpt = wpool.tile([128, 2, 128], f8)
            wet = wpool.tile([128, 2, 128], bf16)
            nc.sync.dma_start(out=wpt[:], in_=wp[:])
            nc.sync.dma_start(out=wet[:], in_=we[:])

            qmap = {"g": nc.gpsimd, "s": nc.sync}
            view_src = {}
            for ui, (lo, n, q) in enumerate(DMA_UNITS):
                xt = xpool.tile([128, n, C, 2, 512], f8, name=f"xu{ui}")
                qmap[q].dma_start(out=xt[:], in_=x[:, lo : lo + n])
                for k in range(n):
                    view_src[lo + k] = (xt, k)

            # ---- PE warmup: plain-mode junk matmuls lift the HAM clock
            # gate (1.2 -> 2.4 GHz) while DMA streams in; the scratch PSUM
            # output is never read.
            warm = psGp.tile([128, 4, 256], f32, name="warm", tag="psg")
            wview = warm[:].rearrange("p a b -> p (a b)")
            for _ in range(N_WARM):
                nc.tensor.matmul(
                    wview[:, 0:512], junk[:, 0:128], junk[:],
                    start=True, stop=True,
                )

            acc = apool.tile([128, 2 * NUNIT], f32)

            sts = {}   # unit k -> stp tile (SBUF S, gh moving operand)

            def emit_gh(k, n):
                stp = sts[k]
                psg = psGp.tile([128, 4, 256], f32, name="psg", tag="psg")
                nc.tensor.matmul(
                    psg[:, 0:n, :], wet[:, 0, :], stp[:, 0:n, 1, :],
                    start=True, stop=True,
                )
                nc.tensor.matmul(
                    psg[:, n : 2 * n, :], wet[:, 1, :], stp[:, 0:n, 0, :],
                    start=True, stop=True,
                )
                hcol = acc[:, NUNIT + k : NUNIT + k + 1]
                psgf = psg[:].rearrange("p a b -> p (a b)")[:, 0 : n * 512]
                if GHABS_ENG[k] == "a":
                    scg = cpool.tile([128, 1024], bf16, name="scg", tag="scg")
                    nc.scalar.activation(
                        scg[:, 0 : n * 512], psgf, Act.Abs, accum_out=hcol,
                    )
                else:
                    nc.vector.tensor_reduce(
                        hcol, psgf, axis=mybir.AxisListType.X,
                        op=AluOp.add, apply_absolute_value=True,
                    )

            prev = None  # (unit index, n)
            for ki, (lo, n) in enumerate(UNITS):
                # PE: S = sum_c (a_c - b_c) via 3 DoubleRow matmuls per view
                pss = psSp.tile([128, 2, 512], f32, name="pss", tag="pss")
                for vl in range(n):
                    xt, li = view_src[lo + vl]
                    for c in range(C):
                        nc.tensor.matmul(
                            pss[:, vl, :], wpt[:], xt[:, li, c],
                            start=(c == 0), stop=(c == C - 1), perf_mode=DR,
                        )
                # gh of the PREVIOUS unit sits here in PE program order,
                # hiding its S-copy latency (software pipelining)
                if prev is not None:
                    emit_gh(*prev)
                # S copy PSUM f32 -> SBUF bf16 (gh moving operand); flat APs
                stp = spool.tile([128, 2, 2, 256], bf16, name=f"st{ki}", tag="st")
                sts[ki] = stp
                stpf = stp[:].rearrange("p v s w -> p (v s w)")
                pssf = pss[:].rearrange("p v sw -> p (v sw)")
                if COPY_ENG[ki] == "a":
                    nc.scalar.activation(
                        stpf[:, 0 : n * 512], pssf[:, 0 : n * 512], Act.Copy
                    )
                else:
                    nc.vector.tensor_scalar_add(
                        stpf[:, 0 : n * 512], pssf[:, 0 : n * 512], 0.0
                    )
                # gw = S[., w+1] - S[., w-1]: interior diff + |edge| copies
                # (TensorTensor may read at most one PSUM input, so gw
                # reads the SBUF S-copy)
                gwt = gpool.tile([128, 2, 2, 256], bf16, name="gwt", tag="gwt")
                geng = nc.vector if GWSUB_ENG[ki] == "v" else nc.gpsimd
                geng.tensor_tensor(
                    gwt[:, 0:n, :, 0:254], stp[:, 0:n, :, 2:256],
                    stp[:, 0:n, :, 0:254], AluOp.subtract,
                )
                geng.tensor_scalar_add(
                    gwt[:, 0:n, :, 254:256], stp[:, 0:n, :, 1:255:253], 0.0
                )
                wcol = acc[:, ki : ki + 1]
                gwtf = gwt[:].rearrange("p v s w -> p (v s w)")[:, 0 : n * 512]
                if GWABS_ENG[ki] == "a":
                    scr = zpool.tile([128, 1024], bf16, name="scr", tag="scr")
                    nc.scalar.activation(
                        scr[:, 0 : n * 512], gwtf, Act.Abs, accum_out=wcol,
                    )
                elif GWABS_ENG[ki] == "g":
                    # abs_max(x, 0) = |x|; tensor_scalar fuses abs + accum
                    scr = zpool.tile([128, 1024], bf16, name="scr", tag="scr")
                    nc.gpsimd.tensor_scalar(
                        scr[:, 0 : n * 512], gwtf, 0.0, 0.0,
                        op0=AluOp.abs_max, op1=AluOp.add, accum_out=wcol,
                    )
                else:
                    nc.vector.tensor_reduce(
                        wcol, gwtf, axis=mybir.AxisListType.X,
                        op=AluOp.add, apply_absolute_value=True,
                    )
                prev = (ki, n)

            emit_gh(*prev)

            nc.sync.dma_start(out=y[:], in_=acc[:])

    nc.finalize()
    _cache["nc"] = nc
    return nc


def _pack(infer, ref):
    """f32 [2,7,7,3,256,256] x2 -> per-core fp8 [128, V, C, 2, 512] packed."""
    f8 = ml_dtypes.float8_e4m3
    a = np.asarray(infer, dtype=np.float32).reshape(98, C, H, W).astype(f8)
    b = np.asarray(ref, dtype=np.float32).reshape(98, C, H, W).astype(f8)
    pad = np.zeros((6, C, H, W), f8)
    a = np.concatenate([a, pad], axis=0).reshape(104, C, 128, 2, W)
    b = np.concatenate([b, pad], axis=0).reshape(104, C, 128, 2, W)
    X = np.stack([a, b], axis=3)                # [104, C, 128, t, s, W]
    X = X.transpose(2, 0, 1, 3, 4, 5)           # [128, 104, C, t, s, W]
    cores = []
    for i in range(N_CORES):
        xi = np.ascontiguousarray(X[:, i * V : (i + 1) * V])
        cores.append(xi.reshape(128, V, C, 2, 512))
    return cores


def _run(infer, ref, trace=False, trace_kwargs=None):
    nc = _build()
    cores = _pack(infer, ref)
    wpair, weo = _weights()
    wpair = wpair.astype(ml_dtypes.float8_e4m3)
    weo = weo.astype(ml_dtypes.bfloat16)
    in_maps = [
        {"x": cores[i], "wp": wpair, "we": weo} for i in range(N_CORES)
    ]
    kwargs = {}
    if trace:
        kwargs["trace"] = True
        if trace_kwargs:
            kwargs["trace_kwargs"] = trace_kwargs
    out = run_bass_kernel_spmd(nc, in_maps, core_ids=list(range(N_CORES)), **kwargs)
    total = 0.0
    for res in out.results:
        total += res["y"].astype(np.float64).sum()
    loss = np.float32(total * SCALE)
    return loss, out


def kernel(infer, ref):
    loss, _ = _run(infer, ref)
    return np.asarray(loss, dtype=np.float32)


# revision 26
# speedup vs baseline: 1.0357x; 1.0357x over previous
"""Detail-loss kernel for TRN2 (8 NeuronCores), v3.

Reference computation (algebraically reduced):
  views = reshape(inputs, (98, 3, 256, 256)); d = infer - ref
  S[n] = sum_c d[n, c]                       (per-view 256x256 plane)
  loss = ( sum |S[n,h,w+1] - S[n,h,w-1]|     (zero-padded outside)
         + sum |S[n,h+1,w] - S[n,h-1,w]| ) / (4 * 98 * 258 * 256)

Sharding: 98 views padded to 104, 13 views per core (zero views add 0).

v3 changes vs v2 (41.8us):
  * DMA rebalanced: views 0-4 on gpsimd(SWDGE), 5-8 on sync, 9-12 on
    scalar queues as 2-view-group transfers (128 x 6KB descriptors).
    v2 put 9/13 views on gpsimd -> 7us single-queue DMA tail.
  * PE warmup: dummy matmuls on junk data at kernel start lift the
    PE_HAM clock gate (1.2 -> 2.4 GHz) before real data arrives, so
    real matmuls run at the 216ns warm cadence instead of ~430ns.
  * gw computed directly from PSUM S on DVE (interior diff + edge-col
    copy), removing the S-copy -> gw chain; S-copy (needed only as the
    gh matmul moving operand) runs in parallel on ACT.
  * per-pair abs-accumulate ops alternate DVE/ACT; some mid-kernel gw
    subtracts route via SBUF on gpsimd to relieve DVE.
Host: sum partials in float64, scale.
"""
import numpy as np
import ml_dtypes
import concourse.bass as bass
import concourse.mybir as mybir
from concourse import bacc
from concourse.tile import TileContext
from concourse.bass_utils import run_bass_kernel_spmd

N_CORES = 8
V = 13                       # views per core (98 -> 104 padded)
C, H, W = 3, 256, 256
SCALE = 1.0 / (4.0 * 98.0 * 258.0 * 256.0)

# x traffic rides ONE DMA ring (gpsimd/SWDGE) in FIFO order: a single
# ring running alone sustains ~358GB/s (HBM cap), while concurrent rings
# round-robin at packet granularity and all complete late (~260GB/s
# aggregate, no useful ordering). Exception: weights + view 0 go on the
# sync ring (HWDGE, faster first-byte; drains before the gpsimd stream
# ramps) so compute starts ~2us earlier. First views go singly for early
# pipeline start; later ones per-pair (fewer Q7 descriptor-gen ops).
# Units are (view_lo, n_views, queue).
DMA_UNITS = [
    (0, 1, "g"), (1, 1, "g"), (2, 1, "g"), (3, 1, "g"),
    (4, 2, "g"), (6, 2, "g"), (8, 2, "g"), (10, 2, "g"), (12, 1, "g"),
]
# processing units: (view_lo, n)
UNITS = [(0, 1), (1, 1), (2, 1), (3, 1), (4, 2), (6, 2), (8, 2), (10, 2), (12, 1)]
NUNIT = len(UNITS)

N_WARM = 7                   # PE warmup matmuls (512 cols, plain mode)

# engine assignment per unit, from measured per-op costs (gpsimd can run
# only plain TENSOR_TENSOR/TENSOR_SCALAR — no PSUM access, no reduce):
#   S-copy: ACT (~1.1ns/elem); gw-sub: gpsimd once its DMA descriptor
#   generation ends (~13us), DVE for the first and last units; gw-abs:
#   DVE reduce; gh-abs: alternate ACT/DVE (DVE at the tail: no
#   READ_ACCUMULATOR step).
COPY_ENG = "aaaaaaaaa"
GWSUB_ENG = "vvggggggv"
GWABS_ENG = "vvvvvvvvv"
GHABS_ENG = "avavavavv"

_cache = {}


def _weights():
    I = np.eye(128, dtype=np.float32)
    E = (np.eye(128) - np.eye(128, k=1)).astype(np.float32)   # out[o]=in[o]-in[o-1]
    O = (np.eye(128, k=-1) - np.eye(128)).astype(np.float32)  # out[o]=in[o+1]-in[o]
    wpair = np.stack([I, -I], axis=1)  # [128, 2, 128] DoubleRow stationary
    weo = np.stack([E, O], axis=1)     # [128, 2, 128]
    return wpair, weo


def _build():
    if "nc" in _cache:
        return _cache["nc"]
    f32 = mybir.dt.float32
    bf16 = mybir.dt.bfloat16
    f8 = mybir.dt.float8e4
    AluOp = mybir.AluOpType
    Act = mybir.ActivationFunctionType
    DR = mybir.MatmulPerfMode.DoubleRow

    nc = bacc.Bacc(None, target_bir_lowering=False)
    x = nc.declare_dram_parameter("x", [128, V, C, 2, 512], f8, isOutput=False)
    wp = nc.declare_dram_parameter("wp", [128, 2, 128], f8, isOutput=False)
    we = nc.declare_dram_parameter("we", [128, 2, 128], bf16, isOutput=False)
    y = nc.declare_dram_parameter("y", [128, 2 * NUNIT], f32, isOutput=True)

    with TileContext(nc) as tc:
        with (
            tc.tile_pool(name="wpool", bufs=1) as wpool,
            tc.tile_pool(name="jp", bufs=1) as jpool,
            tc.tile_pool(name="xp", bufs=1) as xpool,
            tc.tile_pool(name="sp", bufs=3) as spool,
            tc.tile_pool(name="gp", bufs=2) as gpool,
            tc.tile_pool(name="zp", bufs=2) as zpool,
            tc.tile_pool(name="cp", bufs=2) as cpool,
            tc.tile_pool(name="ap", bufs=1) as apool,
            tc.tile_pool(name="psS", bufs=2, space="PSUM") as psSp,
            tc.tile_pool(name="psG", bufs=2, space="PSUM") as psGp,
        ):
            # ---- junk memset first so PE warmup can start immediately
            junk = jpool.tile([128, 512], f8)
            nc.vector.memset(junk[:], 0.0)

            # ---- ALL DMAs ride the gpsimd ring in FIFO order (weights
            # first, tiny). A second ring steals packet slots round-robin
            # and reorders completions, stalling the FIFO pipeline.
            wpt = wpool.tile([128, 2, 128], f8)
            wet = wpool.tile([128, 2, 128], bf16)
            nc.gpsimd.dma_start(out=wpt[:], in_=wp[:])
            nc.gpsimd.dma_start(out=wet[:], in_=we[:])

            qmap = {"g": nc.gpsimd, "s": nc.sync}
            view_src = {}
            for ui, (lo, n, q) in enumerate(DMA_UNITS):
                xt = xpool.tile([128, n, C, 2, 512], f8, name=f"xu{ui}")
                qmap[q].dma_start(out=xt[:], in_=x[:, lo : lo + n])
                for k in range(n):
                    view_src[lo + k] = (xt, k)

            # ---- PE warmup: plain-mode junk matmuls lift the HAM clock
            # gate (1.2 -> 2.4 GHz) while DMA streams in; the scratch PSUM
            # output is never read.
            warm = psGp.tile([128, 4, 256], f32, name="warm", tag="psg")
            wview = warm[:].rearrange("p a b -> p (a b)")
            for _ in range(N_WARM):
                nc.tensor.matmul(
                    wview[:, 0:512], junk[:, 0:128], junk[:],
                    start=True, stop=True,
                )

            acc = apool.tile([128, 2 * NUNIT], f32)

            sts = {}   # unit k -> stp tile (SBUF S, gh moving operand)

            def emit_gh(k, n):
                stp = sts[k]
                psg = psGp.tile([128, 4, 256], f32, name="psg", tag="psg")
                nc.tensor.matmul(
                    psg[:, 0:n, :], wet[:, 0, :], stp[:, 0:n, 1, :],
                    start=True, stop=True,
                )
                nc.tensor.matmul(
                    psg[:, n : 2 * n, :], wet[:, 1, :], stp[:, 0:n, 0, :],
                    start=True, stop=True,
                )
                hcol = acc[:, NUNIT + k : NUNIT + k + 1]
                psgf = psg[:].rearrange("p a b -> p (a b)")[:, 0 : n * 512]
                if GHABS_ENG[k] == "a":
                    scg = cpool.tile([128, 1024], bf16, name="scg", tag="scg")
                    nc.scalar.activation(
                        scg[:, 0 : n * 512], psgf, Act.Abs, accum_out=hcol,
                    )
                else:
                    nc.vector.tensor_reduce(
                        hcol, psgf, axis=mybir.AxisListType.X,
                        op=AluOp.add, apply_absolute_value=True,
                    )

            prev = None  # (unit index, n)
            for ki, (lo, n) in enumerate(UNITS):
                # PE: S = sum_c (a_c - b_c) via 3 DoubleRow matmuls per view
                pss = psSp.tile([128, 2, 512], f32, name="pss", tag="pss")
                for vl in range(n):
                    xt, li = view_src[lo + vl]
                    for c in range(C):
                        nc.tensor.matmul(
                            pss[:, vl, :], wpt[:], xt[:, li, c],
                            start=(c == 0), stop=(c == C - 1), perf_mode=DR,
                        )
                # gh of the PREVIOUS unit sits here in PE program order,
                # hiding its S-copy latency (software pipelining)
                if prev is not None:
                    emit_gh(*prev)
                # S copy PSUM f32 -> SBUF bf16 (gh moving operand); flat APs
                stp = spool.tile([128, 2, 2, 256], bf16, name=f"st{ki}", tag="st")
                sts[ki] = stp
                stpf = stp[:].rearrange("p v s w -> p (v s w)")
                pssf = pss[:].rearrange("p v sw -> p (v sw)")
                if COPY_ENG[ki] == "a":
                    nc.scalar.activation(
                        stpf[:, 0 : n * 512], pssf[:, 0 : n * 512], Act.Copy
                    )
                else:
                    nc.vector.tensor_scalar_add(
                        stpf[:, 0 : n * 512], pssf[:, 0 : n * 512], 0.0
                    )
                # gw = S[., w+1] - S[., w-1]: interior diff + |edge| copies
                # (TensorTensor may read at most one PSUM input, so gw
                # reads the SBUF S-copy)
                gwt = gpool.tile([128, 2, 2, 256], bf16, name="gwt", tag="gwt")
                geng = nc.vector if GWSUB_ENG[ki] == "v" else nc.gpsimd
                geng.tensor_tensor(
                    gwt[:, 0:n, :, 0:254], stp[:, 0:n, :, 2:256],
                    stp[:, 0:n, :, 0:254], AluOp.subtract,
                )
                # |edge| columns ride on DVE (tiny; runs parallel to a
                # gpsimd sub — disjoint gwt columns)
                nc.vector.tensor_scalar_add(
                    gwt[:, 0:n, :, 254:256], stp[:, 0:n, :, 1:255:253], 0.0
                )
                wcol = acc[:, ki : ki + 1]
                gwtf = gwt[:].rearrange("p v s w -> p (v s w)")[:, 0 : n * 512]
                if GWABS_ENG[ki] == "a":
                    scr = zpool.tile([128, 1024], bf16, name="scr", tag="scr")
                    nc.scalar.activation(
                        scr[:, 0 : n * 512], gwtf, Act.Abs, accum_out=wcol,
                    )
                elif GWABS_ENG[ki] == "g":
                    # abs_max(x, 0) = |x|; tensor_scalar fuses abs + accum
                    scr = zpool.tile([128, 1024], bf16, name="scr", tag="scr")
                    nc.gpsimd.tensor_scalar(
                        scr[:, 0 : n * 512], gwtf, 0.0, 0.0,
                        op0=AluOp.abs_max, op1=AluOp.add, accum_out=wcol,
                    )
                else:
                    nc.vector.tensor_reduce(
                        wcol, gwtf, axis=mybir.AxisListType.X,
                        op=AluOp.add, apply_absolute_value=True,
                    )
                prev = (ki, n)

            emit_gh(*prev)

            nc.sync.dma_start(out=y[:], in_=acc[:])

    nc.finalize()
    _cache["nc"] = nc
    return nc


def _pack(infer, ref):
    """f32 [2,7,7,3,256,256] x2 -> per-core fp8 [128, V, C, 2, 512] packed."""
    f8 = ml_dtypes.float8_e4m3
    a = np.asarray(infer, dtype=np.float32).reshape(98, C, H, W).astype(f8)
    b = np.asarray(ref, dtype=np.float32).reshape(98, C, H, W).astype(f8)
    pad = np.zeros((6, C, H, W), f8)
    a = np.concatenate([a, pad], axis=0).reshape(104, C, 128, 2, W)
    b = np.concatenate([b, pad], axis=0).reshape(104, C, 128, 2, W)
    X = np.stack([a, b], axis=3)                # [104, C, 128, t, s, W]
    X = X.transpose(2, 0, 1, 3, 4, 5)           # [128, 104, C, t, s, W]
    cores = []
    for i in range(N_CORES):
        xi = np.ascontiguousarray(X[:, i * V : (i + 1) * V])
        cores.append(xi.reshape(128, V, C, 2, 512))
    return cores


def _run(infer, ref, trace=False, trace_kwargs=None):
    nc = _build()
    cores = _pack(infer, ref)
    wpair, weo = _weights()
    wpair = wpair.astype(ml_dtypes.float8_e4m3)
    weo = weo.astype(ml_dtypes.bfloat16)
    in_maps = [
        {"x": cores[i], "wp": wpair, "we": weo} for i in range(N_CORES)
    ]
    kwargs = {}
    if trace:
        kwargs["trace"] = True
        if trace_kwargs:
            kwargs["trace_kwargs"] = trace_kwargs
    out = run_bass_kernel_spmd(nc, in_maps, core_ids=list(range(N_CORES)), **kwargs)
    total = 0.0
    for res in out.results:
        total += res["y"].astype(np.float64).sum()
    loss = np.float32(total * SCALE)
    return loss, out


def kernel(infer, ref):
    loss, _ = _run(infer, ref)
    return np.asarray(loss, dtype=np.float32)
